# revision 1
# baseline (speedup 1.0000x reference)
"""WENO5 2D advection tendency kernel for 8 Trainium2 NeuronCores.

Strategy
  * rows sharded 256/core, halos materialized host-side (no collectives)
  * x-flux: y-major tiles [128 rows, xc+6]; all stencil shifts on the free dim
  * y-flux: x-major tiles [128 cols, macb, rows+6] built by PE transposes;
    the y divergence is transposed back through PSUM and fused into the
    final add (compute engines cannot read partition-shifted operands, so
    both passes keep their stencil axis on the free dimension)
  * WENO5 refactored so both upwind reconstructions share one set of
    smoothness indicators (mirrored-stencil betas are shifted views of the
    forward ones); everything lowers to 2-input fused custom DVE ops.
"""

import math

import numpy as np

import concourse.bass as bass
import concourse.bacc as bacc
import concourse.mybir as mybir
from concourse.tile import TileContext

F32 = mybir.dt.float32
AF = mybir.ActivationFunctionType
MUL = mybir.AluOpType.mult
ADD = mybir.AluOpType.add

DX = 1000.0
DY = 1000.0
EPS = 1e-8
K1 = 13.0 / 12.0
K2 = 0.25

NY, NX = 2048, 4096
N_CORES = 8
RPC = NY // N_CORES  # rows per core
XC = 1024            # x-chunk width for the x-pass
MACB = 4             # x-blocks of 128 per y-pass macro tile

# --------------------------------------------------------------------------
# Custom DVE ops (registered at import). Specs avoid imm2/C2 so they stay
# legal when in1 is a 3D access pattern (y-pass tiles are [128, B, W]).
# --------------------------------------------------------------------------
from concourse.dve_ops import (
    OPS,
    CUSTOM_DVE_SPECS,
    DveOp,
    _SUB_OPCODE_FOR_NAME,
)
from concourse.dve_spec import (
    Spec,
    Src0,
    Src1,
    C0,
    C1,
    Zero,
    lower,
    minn,
    relu,
    sq,
    _has_src1,
)
from concourse.dve_uop import DveOpSpec


def _register_op(name, body, reference):
    for op in OPS:
        if op.name == name:
            return op
    spec = Spec(body=body, reference=reference)
    if name not in _SUB_OPCODE_FOR_NAME:
        _SUB_OPCODE_FOR_NAME[name] = max(_SUB_OPCODE_FOR_NAME.values()) + 1
    opcode = _SUB_OPCODE_FOR_NAME[name]
    shas = {}
    for ver in ("v3", "v4"):
        uops = lower(spec, ver=ver)
        shas[ver] = DveOpSpec(
            name=name, opcode=opcode, uops=uops, rd1_en=_has_src1(spec)
        ).sha(ver)
    op = DveOp(name, spec, subdim=False, uops_sha=shas)
    OPS.append(op)
    CUSTOM_DVE_SPECS[name] = spec
    return op




def _f2(a):
    """Flatten free dims (CoreSim may pass coalesced vs non-coalesced APs)."""
    return a.reshape(a.shape[0], -1)

OP_SQD = _register_op(  # sq(s0*a - b)
    "ANT_SQD",
    sq(Src0 * C0 - Src1),
    lambda in0, in1, s0, s1, imm2: np.square(_f2(in0) * s0 - _f2(in1)),
)
OP_SQA = _register_op(  # sq(s0*a + b + s1)
    "ANT_SQA",
    sq(Src0 * C0 + Src1 + C1),
    lambda in0, in1, s0, s1, imm2: np.square(_f2(in0) * s0 + _f2(in1) + s1),
)
OP_LSC = _register_op(  # s1*(s0*a - b)
    "ANT_LSC",
    (Src0 * C0 - Src1) * C1,
    lambda in0, in1, s0, s1, imm2: (_f2(in0) * s0 - _f2(in1)) * s1,
)
OP_RELUMUL = _register_op(  # relu(a)*b
    "ANT_RELUMUL",
    relu(Src0) * Src1,
    lambda in0, in1, s0, s1, imm2: np.maximum(_f2(in0), 0.0) * _f2(in1),
)
OP_MINMUL = _register_op(  # min(a,0)*b
    "ANT_MINMUL",
    minn(Src0, Zero) * Src1,
    lambda in0, in1, s0, s1, imm2: np.minimum(_f2(in0), 0.0) * _f2(in1),
)




# --------------------------------------------------------------------------
# Device program helpers
# --------------------------------------------------------------------------
class _Arr:
    """A tile plus the global index its local free element 0 maps to."""

    def __init__(self, tile, base):
        self.tile = tile
        self.base = base

    def full(self):
        return self.tile[:]

    def win(self, g0, w):
        o = g0 - self.base
        ap = self.tile
        assert o >= 0, (self.base, g0)
        if len(ap.shape) == 2:
            return ap[:, o:o + w]
        return ap[:, :, o:o + w]


class _Slots:
    """Manual lifetime manager: a small set of rotating pool tags."""

    def __init__(self, pool, shape3, nslots=14, prefix="w", bufs=None):
        self.pool = pool
        self.shape3 = shape3
        self.bufs = bufs
        self.free_tags = [f"{prefix}{i}" for i in range(nslots)]
        self.tags = {}

    def alloc(self, name, w, base):
        tag = self.free_tags.pop()
        s = list(self.shape3)
        s[-1] = w
        t = self.pool.tile(s, F32, tag=tag, name=f"{tag}_{name}",
                           bufs=self.bufs)
        self.tags[name] = tag
        return _Arr(t, base)

    def free(self, *names):
        for n in names:
            self.free_tags.append(self.tags.pop(n))


def _flux_ops(nc, sl, H, U, F0, WF, NF):
    """Fused WENO5 upwind flux for one direction.

    H: cell-indexed field _Arr; U: face-indexed velocity _Arr.
    F0: first face; WF = NF + 1 (face arrays carry one extra face so
    [f+1] shifts exist); returns flux _Arr over faces [F0, F0+NF).
    """
    V = nc.vector
    A = nc.scalar

    ncell = WF + 3
    dq = sl.alloc("dq", ncell, F0 - 2)
    V.tensor_sub(dq.full(), H.win(F0 - 1, ncell), H.win(F0 - 2, ncell))

    nd2 = WF + 2
    d2 = sl.alloc("d2", nd2, F0 - 1)
    V.tensor_sub(d2.full(), dq.win(F0 - 1, nd2), dq.win(F0 - 2, nd2))

    d2sq = sl.alloc("d2sq", nd2, F0 - 1)  # k1*d2^2 on ACT
    A.activation(d2sq.full(), d2.full(), AF.Square, scale=math.sqrt(K1))

    # tF = (1/6)dq[f-1] + (1/3)dq[f];  tG = -(1/6)dq[f+1] - (1/3)dq[f]
    tF = sl.alloc("tf", NF, F0)
    V._custom_dve(OP_LSC, out=tF.full(), in0=dq.win(F0 - 1, NF),
                  in1=dq.win(F0, NF), s0=-0.5, s1=-1.0 / 3.0)
    tG = sl.alloc("tg", NF, F0)
    V._custom_dve(OP_LSC, out=tG.full(), in0=dq.win(F0 + 1, NF),
                  in1=dq.win(F0, NF), s0=-0.5, s1=1.0 / 3.0)

    X2 = sl.alloc("x2", WF, F0)
    V._custom_dve(OP_SQD, out=X2.full(), in0=dq.win(F0 - 1, WF),
                  in1=dq.win(F0 - 2, WF), s0=3.0)
    X3 = sl.alloc("x3", WF, F0)
    V._custom_dve(OP_SQD, out=X3.full(), in0=dq.win(F0 - 1, WF),
                  in1=dq.win(F0, WF), s0=-1.0)
    X4 = sl.alloc("x4", WF, F0)
    V._custom_dve(OP_SQD, out=X4.full(), in0=dq.win(F0, WF),
                  in1=dq.win(F0 + 1, WF), s0=3.0)
    sl.free("dq")

    nwd = WF + 1
    Wd = sl.alloc("wd", nwd, F0 - 1)  # Wd[f] = d2[f] - d2[f+1]
    V.tensor_sub(Wd.full(), d2.win(F0 - 1, nwd), d2.win(F0, nwd))
    sl.free("d2")

    B1 = sl.alloc("b1", WF, F0)
    V._custom_dve(OP_SQA, out=B1.full(), in0=X2.full(),
                  in1=d2sq.win(F0 - 1, WF), s0=K2, s1=EPS)
    B2 = sl.alloc("b2", WF, F0)
    V._custom_dve(OP_SQA, out=B2.full(), in0=X3.full(),
                  in1=d2sq.win(F0, WF), s0=K2, s1=EPS)
    B3 = sl.alloc("b3", WF, F0)
    V._custom_dve(OP_SQA, out=B3.full(), in0=X4.full(),
                  in1=d2sq.win(F0 + 1, WF), s0=K2, s1=EPS)
    sl.free("x2", "x3", "x4", "d2sq")

    p12 = sl.alloc("p12", WF, F0)
    V.tensor_mul(p12.full(), B1.full(), B2.full())
    p13 = sl.alloc("p13", WF, F0)
    V.tensor_mul(p13.full(), B1.full(), B3.full())
    p23 = sl.alloc("p23", WF, F0)
    V.tensor_mul(p23.full(), B2.full(), B3.full())
    sl.free("b1", "b2", "b3")

    # denominators (x10): denL = p23 + 6 p13 + 3 p12; denR mirrored (@f+1)
    uL = sl.alloc("ul", WF, F0)
    V.scalar_tensor_tensor(uL.full(), p13.full(), 2.0, p12.full(), MUL, ADD)
    uR = sl.alloc("ur", WF, F0)
    V.scalar_tensor_tensor(uR.full(), p13.full(), 2.0, p23.full(), MUL, ADD)
    sl.free("p13")
    denL = sl.alloc("denl", WF, F0)
    V.scalar_tensor_tensor(denL.full(), uL.full(), 3.0, p23.full(), MUL, ADD)
    denR = sl.alloc("denr", WF, F0)
    V.scalar_tensor_tensor(denR.full(), uR.full(), 3.0, p12.full(), MUL, ADD)
    sl.free("ul", "ur")

    rL = sl.alloc("rl", WF, F0)
    V.reciprocal_approx_fast(out=rL.full(), in_=denL.full())
    rR = sl.alloc("rr", WF, F0)
    V.reciprocal_approx_fast(out=rR.full(), in_=denR.full())
    sl.free("denl", "denr")

    PA = sl.alloc("pa", WF, F0)
    V.tensor_mul(PA.full(), p23.full(), Wd.win(F0 - 1, WF))
    PB = sl.alloc("pb", WF, F0)
    V.tensor_mul(PB.full(), p12.full(), Wd.win(F0, WF))
    sl.free("p12", "p23", "wd")

    numL = sl.alloc("numl", WF, F0)
    V.scalar_tensor_tensor(numL.full(), PB.full(), 1.5, PA.full(), MUL, ADD)
    numR = sl.alloc("numr", WF, F0)
    V.scalar_tensor_tensor(numR.full(), PA.full(), 1.5, PB.full(), MUL, ADD)
    sl.free("pa", "pb")

    mL = sl.alloc("ml", WF, F0)
    V.tensor_mul(mL.full(), numL.full(), rL.full())
    mR = sl.alloc("mr", WF, F0)
    V.tensor_mul(mR.full(), numR.full(), rR.full())
    sl.free("numl", "numr", "rl", "rr")

    WL = sl.alloc("wl", NF, F0)
    V.scalar_tensor_tensor(WL.full(), mL.win(F0, NF), 1.0 / 3.0,
                           tF.full(), MUL, ADD)
    WR = sl.alloc("wr", NF, F0)
    V.scalar_tensor_tensor(WR.full(), mR.win(F0 + 1, NF), -1.0 / 3.0,
                           tG.full(), MUL, ADD)
    sl.free("ml", "mr", "tf", "tg")

    qL = sl.alloc("ql", NF, F0)
    V.tensor_add(qL.full(), WL.full(), H.win(F0, NF))
    qR = sl.alloc("qr", NF, F0)
    V.tensor_add(qR.full(), WR.full(), H.win(F0 + 1, NF))
    sl.free("wl", "wr")

    t1 = sl.alloc("t1", NF, F0)
    V._custom_dve(OP_RELUMUL, out=t1.full(), in0=U.win(F0, NF), in1=qL.full())
    t2 = sl.alloc("t2", NF, F0)
    V._custom_dve(OP_MINMUL, out=t2.full(), in0=U.win(F0, NF), in1=qR.full())
    sl.free("ql", "qr")

    fe = sl.alloc("fe", NF, F0)
    V.tensor_add(fe.full(), t1.full(), t2.full())
    sl.free("t1", "t2")
    return fe


def build_program(rpc=RPC, nx=NX, xc=XC, macb=MACB, parts="xyt",
                  work_bufs=2, nslots=12, reps=1):
    """SPMD Bass program computing one core's [rpc, nx] tendency block."""
    assert rpc % 128 == 0 and nx % xc == 0 and nx % (128 * macb) == 0
    assert xc % 128 == 0
    yb = rpc // 128
    nchunk = nx // xc
    nmac = nx // (128 * macb)
    YW = rpc + 6
    VW = rpc + 1

    nc = bacc.Bacc("TRN2", target_bir_lowering=False, debug=False)
    hs = nc.dram_tensor("hs", [rpc + 6, nx + 6], F32, kind="ExternalInput")
    us = nc.dram_tensor("us", [rpc, nx + 6], F32, kind="ExternalInput")
    vs = nc.dram_tensor("vs", [rpc + 1, nx], F32, kind="ExternalInput")
    out_d = nc.dram_tensor("out", [rpc, nx], F32, kind="ExternalOutput")

    V = nc.vector
    A = nc.scalar

    with TileContext(nc) as tc:
        with (
            tc.tile_pool(name="const", bufs=1) as cpool,
            tc.tile_pool(name="io", bufs=2) as io,
            tc.tile_pool(name="ysrc", bufs=4) as ysrc,
            tc.tile_pool(name="work", bufs=1) as work,
            tc.tile_pool(name="zxkeep", bufs=1) as zxkeep,
            tc.tile_pool(name="outp", bufs=3) as outp,
            tc.tile_pool(name="psum", bufs=2, space="PSUM") as pps,
        ):
            from concourse.masks import make_identity
            ident = cpool.tile([128, 128], F32, name="ident")
            make_identity(nc, ident[:])

            for _rep in range(reps):
              zx_arrs = {}

              # ---------------- x-pass ----------------
              for b in range(yb) if "x" in parts else []:
                  for c in range(nchunk):
                      F0 = c * xc - 1
                      WF = xc + 2
                      bH = c * xc - 3
                      HX = _Arr(io.tile([128, xc + 6], F32, tag="hx", name="hx"), bH)
                      nc.sync.dma_start(
                          out=HX.full(),
                          in_=hs[3 + b * 128:3 + b * 128 + 128,
                                 c * xc:c * xc + xc + 6],
                      )
                      UX = _Arr(io.tile([128, xc + 1], F32, tag="ux", name="ux"), F0)
                      nc.sync.dma_start(
                          out=UX.full(),
                          in_=us[b * 128:b * 128 + 128,
                                 c * xc + 2:c * xc + xc + 3],
                      )
                      sl = _Slots(work, [128, 0], nslots=nslots, bufs=work_bufs)
                      fe = _flux_ops(nc, sl, HX, UX, F0, WF, xc + 1)
                      zx = _Arr(zxkeep.tile([128, xc], F32, tag=f"zx{b}_{c}", name=f"zx{b}_{c}"),
                                c * xc)
                      V._custom_dve(OP_LSC, out=zx.full(),
                                    in0=fe.win(c * xc, xc),
                                    in1=fe.win(c * xc - 1, xc),
                                    s0=1.0, s1=-1.0 / DX)
                      sl.free("fe")
                      zx_arrs[(b, c)] = zx
                      if "y" not in parts and "t" not in parts:
                          nc.sync.dma_start(
                              out=out_d[b * 128:b * 128 + 128, c * xc:(c + 1) * xc],
                              in_=zx.full())

              # ---------------- y-pass ----------------
              for mac in range(nmac) if ("y" in parts or "t" in parts) else []:
                  HT = io.tile([128, macb, YW], F32, tag="ht", name="ht")
                  VT = io.tile([128, macb, VW], F32, tag="vt", name="vt")
                  for bb in range(macb):
                      x0 = (mac * macb + bb) * 128
                      hp = pps.tile([128, YW], F32, tag="hps", name="hps")
                      for roff in range(0, YW, 128):
                          rlen = min(128, YW - roff)
                          srct = ysrc.tile([128, 128], F32, tag="hsrc", name="hsrc")
                          nc.sync.dma_start(
                              out=srct[:rlen, :],
                              in_=hs[roff:roff + rlen, 3 + x0:3 + x0 + 128],
                          )
                          nc.tensor.transpose(
                              hp[:, roff:roff + rlen], srct[:rlen, :],
                              ident[:rlen, :rlen],
                          )
                      A.copy(HT[:, bb, :], hp[:])
                      vp = pps.tile([128, VW], F32, tag="vps", name="vps")
                      for roff in range(0, VW, 128):
                          rlen = min(128, VW - roff)
                          srct = ysrc.tile([128, 128], F32, tag="vsrc", name="vsrc")
                          nc.sync.dma_start(
                              out=srct[:rlen, :],
                              in_=vs[roff:roff + rlen, x0:x0 + 128],
                          )
                          nc.tensor.transpose(
                              vp[:, roff:roff + rlen], srct[:rlen, :],
                              ident[:rlen, :rlen],
                          )
                      A.copy(VT[:, bb, :], vp[:])

                  if "y" not in parts:
                      # debug: dump HT slice to out and continue
                      dump = outp.tile([128, macb * 128], F32, tag="outsb", name="dump")
                      for b in range(yb):
                          for bb in range(macb):
                              V.tensor_copy(out=dump[:, bb * 128:bb * 128 + 128],
                                            in_=HT[:, bb, b * 128:b * 128 + 128])
                          nc.sync.dma_start(
                              out=out_d[b * 128:b * 128 + 128,
                                        mac * macb * 128:(mac + 1) * macb * 128],
                              in_=dump[:])
                      continue
                  # face tf corresponds to row r0-1+tf; cell j -> HT[..., j+2]
                  Hy = _Arr(HT, -2)
                  Uy = _Arr(VT, 0)
                  sl = _Slots(work, [128, macb, 0], nslots=nslots, bufs=work_bufs)
                  fn = _flux_ops(nc, sl, Hy, Uy, 0, rpc + 2, rpc + 1)
                  zyT = work.tile([128, macb, rpc], F32, tag="zyt", name="zyt")
                  V._custom_dve(OP_LSC, out=zyT[:],
                                in0=fn.win(1, rpc), in1=fn.win(0, rpc),
                                s0=1.0, s1=-1.0 / DY)
                  sl.free("fe")

                  for b in range(yb):
                      ot = outp.tile([128, macb * 128], F32, tag="outsb", name="outsb")
                      for bb in range(macb):
                          zp = pps.tile([128, 128], F32, tag="zyps", name="zyps")
                          nc.tensor.transpose(
                              zp[:], zyT[:, bb, b * 128:b * 128 + 128],
                              ident[:],
                          )
                          g = mac * macb + bb
                          c = (g * 128) // xc
                          if (b, c) in zx_arrs:
                              zx = zx_arrs[(b, c)]
                              V.tensor_add(ot[:, bb * 128:bb * 128 + 128],
                                           zx.win(g * 128, 128), zp[:])
                          else:
                              V.tensor_copy(out=ot[:, bb * 128:bb * 128 + 128],
                                            in_=zp[:])
                      nc.sync.dma_start(
                          out=out_d[b * 128:b * 128 + 128,
                                    mac * macb * 128:(mac + 1) * macb * 128],
                          in_=ot[:],
                      )
    nc.compile()
    return nc


# --------------------------------------------------------------------------
# Host side
# --------------------------------------------------------------------------
def make_shards(h, u, v, n_cores=N_CORES):
    rpc = h.shape[0] // n_cores
    hp = np.pad(h, ((3, 3), (3, 3)), mode="edge")
    up = np.pad(u, ((0, 0), (3, 3)), mode="edge")
    vp = np.pad(v, ((3, 3), (0, 0)), mode="edge")
    maps = []
    for i in range(n_cores):
        r0 = i * rpc
        maps.append({
            "hs": np.ascontiguousarray(hp[r0:r0 + rpc + 6, :]),
            "us": np.ascontiguousarray(up[r0:r0 + rpc, :]),
            "vs": np.ascontiguousarray(vp[r0 + 2:r0 + 2 + rpc + 1, :]),
        })
    return maps


_NC_CACHE = {}


def kernel(h, u, v):
    h = np.asarray(h, dtype=np.float32)
    u = np.asarray(u, dtype=np.float32)
    v = np.asarray(v, dtype=np.float32)
    assert h.shape == (NY, NX), h.shape

    from concourse.bass_utils import run_bass_kernel_spmd

    if "main" not in _NC_CACHE:
        _NC_CACHE["main"] = build_program()
    nc = _NC_CACHE["main"]

    in_maps = make_shards(h, u, v)
    res = run_bass_kernel_spmd(nc, in_maps, list(range(N_CORES)))
    out = np.concatenate([res.results[i]["out"] for i in range(N_CORES)],
                         axis=0)
    out[:2, :] = 0.0
    out[-2:, :] = 0.0
    out[:, :2] = 0.0
    out[:, -2:] = 0.0
    return out



# revision 5
# speedup vs baseline: 15.9769x; 15.9769x over previous
"""WENO5 2D advection tendency kernel v2 for 8 Trainium2 NeuronCores.

Strategy (v2, bf16 multi-engine):
  * rows sharded 256/core, halos materialized host-side (no collectives)
  * all WENO math expressed as scalar_tensor_tensor (TensorScalarPtr) ops in
    bf16 SBUF -> 4x DVE perf mode (0.26 ns/col vs 1.04 for fp32/custom ops)
  * work split across DVE / Pool(GPSIMD) / Act engines via an assignment
    table; squares go to Act (Square activation with folded scale)
  * reciprocal customs replaced by direct TensorTensor divide (bf16 2x)
  * eps-guard: B = (Y + 2eps)*Y = (Y+eps)^2 - eps^2; exact zeros only occur
    in edge-replicated ghost stencils whose outputs the host zeroes anyway
  * x-flux: y-major tiles [128, xc+6]; stencil shifts on the free dim
  * y-flux: x-major tiles built by PE transposes (fp32 in, bf16 evac casts);
    the final out = -(dfx/DX + dfy/DY) is assembled in PSUM by PE:
    transpose-matmul of dfy + identity-matmul accumulate of dfx
  * velocity upwind split up=relu(u), um=min(u,0) pre-scaled by -1/D so the
    flux and divergence come out final-scaled
"""

import math

import numpy as np

import concourse.bass as bass
import concourse.bacc as bacc
import concourse.mybir as mybir
from concourse.tile import TileContext

F32 = mybir.dt.float32
BF16 = mybir.dt.bfloat16
AF = mybir.ActivationFunctionType
MUL = mybir.AluOpType.mult
ADD = mybir.AluOpType.add
SUB = mybir.AluOpType.subtract
DIV = mybir.AluOpType.divide
MAX = mybir.AluOpType.max
MIN = mybir.AluOpType.min

DX = 1000.0
DY = 1000.0
EPS = 1e-8
K1 = 13.0 / 12.0
K2 = 0.25

NY, NX = 2048, 4096
N_CORES = 8
RPC = NY // N_CORES  # rows per core
XC = 1024            # x-chunk width for the x-pass
MACB = 4             # x-blocks of 128 per y-pass macro tile


class _Arr:
    """A tile plus the global index its local free element 0 maps to."""

    def __init__(self, tile, base):
        self.tile = tile
        self.base = base

    def full(self):
        return self.tile[:]

    def win(self, g0, w):
        o = g0 - self.base
        ap = self.tile
        assert o >= 0, (self.base, g0)
        if len(ap.shape) == 2:
            return ap[:, o:o + w]
        return ap[:, :, o:o + w]


class _Slots:
    """Manual lifetime manager: a small set of rotating pool tags."""

    def __init__(self, pool, shape3, nslots=14, prefix="w", bufs=None,
                 dtype=BF16):
        self.pool = pool
        self.shape3 = shape3
        self.bufs = bufs
        self.dtype = dtype
        self.free_tags = [f"{prefix}{i}" for i in range(nslots)]
        self.tags = {}

    def alloc(self, name, w, base, dtype=None):
        tag = self.free_tags.pop()
        s = list(self.shape3)
        s[-1] = w
        t = self.pool.tile(s, dtype or self.dtype, tag=tag,
                           name=f"{tag}_{name}", bufs=self.bufs)
        self.tags[name] = tag
        return _Arr(t, base)

    def alloc_f32(self, name, w, base):
        return self.alloc(name, w, base, dtype=F32)

    def free(self, *names):
        for n in names:
            self.free_tags.append(self.tags.pop(n))


# Engine assignment for the wide flux ops. 'v' = DVE, 'p' = Pool/GPSIMD,
# 'a' = Act. 2-tensor TT ops: DVE 0.52 ns/col (bf16 2x_1p) / Pool 1.98;
# 1-tensor TS ops: DVE 0.26 (4x_2p) / Act 0.833; squares on Act 0.833;
# STT chains (2 tensors + scalar): Pool 1.39 / DVE 1.04.
SK2 = math.sqrt(K2 / K1)   # beta scale with K1 divided out (ratio-invariant)
EPSP = EPS / K1
DEFAULT_ASG = {
    "dq": "v", "dqa": "a", "dqb": "a", "dq6": "a", "dq3": "a",
    "d2": "v", "d2sq": "a",
    "l2": "v", "l3": "p", "l4": "v",
    "x2": "a", "x3": "a", "x4": "a",
    "y1": "v", "y2": "p", "y3": "v",
    "b1": "a", "b2": "a", "b3": "a",
    "wd": "v",
    "p12": "v", "p13": "v", "p23": "v",
    "p23s": "v", "p12s": "v", "p13s": "v",
    "u1": "p", "denl": "p", "u2": "p", "denr": "p",
    "pa": "v", "pb": "v", "pa15": "v", "pb15": "v",
    "numl": "v", "numr": "v",
    "gl": "v", "gr": "v",
    "tf": "v", "tg": "p",
    "al": "v", "ql": "v", "ar": "v", "qr": "v",
    "t1": "v", "t2": "v", "fe": "v",
}


def _flux_v2(nc, sl, H, UP, UM, F0, WF, NF, asg):
    """Decomposed WENO5 upwind flux, pre-scaled by the UP/UM factors.

    H: bf16 cell-indexed _Arr; UP/UM: bf16 relu(u)*c / min(u,0)*c _Arrs.
    Returns flux*c _Arr over faces [F0, F0+NF).
    """
    E = {"v": nc.vector, "p": nc.gpsimd, "a": nc.scalar}
    A = nc.scalar

    def tt(name, out, in0, in1, op):
        E[asg[name]].tensor_tensor(out, in0, in1, op)

    def ts(name, out, in0, s):  # out = in0 * s
        if asg[name] == "a":
            A.activation(out, in0, AF.Copy, scale=float(s))
        else:
            E[asg[name]].tensor_scalar(out, in0, float(s), None, MUL)

    def stt(name, out, in0, s, in1, op0, op1):
        E[asg[name]].scalar_tensor_tensor(out, in0, s, in1, op0, op1)

    def sq(name, out, in0):  # out = in0^2
        if asg[name] == "a":
            A.activation(out, in0, AF.Square)
        else:
            E[asg[name]].tensor_tensor(out, in0, in0, MUL)

    ncell = WF + 3
    dq = sl.alloc("dq", ncell, F0 - 2)
    tt("dq", dq.full(), H.win(F0 - 1, ncell), H.win(F0 - 2, ncell), SUB)

    # pre-scaled dq copies (shared by several linear combos)
    dqa = sl.alloc("dqa", ncell, F0 - 2)   # 3*sk*dq
    ts("dqa", dqa.full(), dq.full(), 3.0 * SK2)
    dqb = sl.alloc("dqb", ncell, F0 - 2)   # sk*dq
    ts("dqb", dqb.full(), dq.full(), SK2)
    dq6 = sl.alloc("dq6", ncell, F0 - 2)   # dq/6
    ts("dq6", dq6.full(), dq.full(), 1.0 / 6.0)
    dq3 = sl.alloc("dq3", ncell, F0 - 2)   # dq/3
    ts("dq3", dq3.full(), dq.full(), 1.0 / 3.0)

    nd2 = WF + 2
    d2 = sl.alloc("d2", nd2, F0 - 1)
    tt("d2", d2.full(), dq.win(F0 - 1, nd2), dq.win(F0 - 2, nd2), SUB)
    sl.free("dq")

    d2sq = sl.alloc("d2sq", nd2, F0 - 1)  # d2^2 (K1 folded out)
    sq("d2sq", d2sq.full(), d2.full())

    # scaled linear beta combos: l = sk * (stencil); X = l^2
    l2 = sl.alloc("l2", WF, F0)
    tt("l2", l2.full(), dqa.win(F0 - 1, WF), dqb.win(F0 - 2, WF), SUB)
    l3 = sl.alloc("l3", WF, F0)
    tt("l3", l3.full(), dqb.win(F0 - 1, WF), dqb.win(F0, WF), ADD)
    l4 = sl.alloc("l4", WF, F0)
    tt("l4", l4.full(), dqa.win(F0, WF), dqb.win(F0 + 1, WF), SUB)
    sl.free("dqa", "dqb")

    X2 = sl.alloc("x2", WF, F0)
    sq("x2", X2.full(), l2.full())
    X3 = sl.alloc("x3", WF, F0)
    sq("x3", X3.full(), l3.full())
    X4 = sl.alloc("x4", WF, F0)
    sq("x4", X4.full(), l4.full())
    sl.free("l2", "l3", "l4")

    nwd = WF + 1
    Wd = sl.alloc("wd", nwd, F0 - 1)  # Wd[f] = d2[f] - d2[f+1]
    tt("wd", Wd.full(), d2.win(F0 - 1, nwd), d2.win(F0, nwd), SUB)
    sl.free("d2")

    # Y = X + d2sq(shift);  B = (Y + eps')^2
    Y1 = sl.alloc("y1", WF, F0)
    tt("y1", Y1.full(), X2.full(), d2sq.win(F0 - 1, WF), ADD)
    Y2 = sl.alloc("y2", WF, F0)
    tt("y2", Y2.full(), X3.full(), d2sq.win(F0, WF), ADD)
    Y3 = sl.alloc("y3", WF, F0)
    tt("y3", Y3.full(), X4.full(), d2sq.win(F0 + 1, WF), ADD)
    sl.free("x2", "x3", "x4", "d2sq")

    def beta(name, y_arr):
        b = sl.alloc(name, WF, F0)
        if asg[name] == "a":
            A.activation(b.full(), y_arr.full(), AF.Square, bias=EPSP)
        else:
            ye = sl.alloc(name + "e", WF, F0)
            E[asg[name]].tensor_scalar(ye.full(), y_arr.full(), EPSP, None,
                                       ADD)
            E[asg[name]].tensor_tensor(b.full(), ye.full(), ye.full(), MUL)
            sl.free(name + "e")
        return b

    B1 = beta("b1", Y1)
    B2 = beta("b2", Y2)
    B3 = beta("b3", Y3)
    sl.free("y1", "y2", "y3")

    p12 = sl.alloc("p12", WF, F0)
    tt("p12", p12.full(), B1.full(), B2.full(), MUL)
    p13 = sl.alloc("p13", WF, F0)
    tt("p13", p13.full(), B1.full(), B3.full(), MUL)
    p23 = sl.alloc("p23", WF, F0)
    tt("p23", p23.full(), B2.full(), B3.full(), MUL)
    sl.free("b1", "b2", "b3")

    # den (x30): denL3 = 3 p23 + 18 p13 + 9 p12; denR3 mirrored.
    # TS pre-scales on DVE (0.26), wide adds on Pool TT (GPSIMD supports
    # only TensorTensor add/mult; TensorScalarPtr fails codegen there).
    p13_18 = sl.alloc("p13_18", WF, F0)
    ts("p13s", p13_18.full(), p13.full(), 18.0)
    p12_9 = sl.alloc("p12_9", WF, F0)
    ts("p12s", p12_9.full(), p12.full(), 9.0)
    p23_9 = sl.alloc("p23_9", WF, F0)
    ts("p23s", p23_9.full(), p23.full(), 9.0)
    p12_3 = sl.alloc("p12_3", WF, F0)
    ts("p12s", p12_3.full(), p12.full(), 3.0)
    p23_3 = sl.alloc("p23_3", WF, F0)
    ts("p23s", p23_3.full(), p23.full(), 3.0)
    a1 = sl.alloc("a1", WF, F0)
    tt("u1", a1.full(), p13_18.full(), p12_9.full(), ADD)
    denL = sl.alloc_f32("denl", WF, F0)
    tt("denl", denL.full(), a1.full(), p23_3.full(), ADD)
    a2 = sl.alloc("a2", WF, F0)
    tt("u2", a2.full(), p13_18.full(), p23_9.full(), ADD)
    denR = sl.alloc_f32("denr", WF, F0)
    tt("denr", denR.full(), a2.full(), p12_3.full(), ADD)
    sl.free("a1", "a2", "p13", "p13_18", "p12_9", "p23_9", "p12_3",
            "p23_3")

    PA = sl.alloc("pa", WF, F0)
    tt("pa", PA.full(), p23.full(), Wd.win(F0 - 1, WF), MUL)
    PB = sl.alloc("pb", WF, F0)
    tt("pb", PB.full(), p12.full(), Wd.win(F0, WF), MUL)
    sl.free("p12", "p23", "wd")

    # numL = PA + 1.5 PB ; numR = PB + 1.5 PA  (gL = numL/denL3 = mL/3)
    pa15 = sl.alloc("pa15", WF, F0)
    ts("pa15", pa15.full(), PA.full(), 1.5)
    pb15 = sl.alloc("pb15", WF, F0)
    ts("pb15", pb15.full(), PB.full(), 1.5)
    numL = sl.alloc("numl", WF, F0)
    tt("numl", numL.full(), PA.full(), pb15.full(), ADD)
    numR = sl.alloc("numr", WF, F0)
    tt("numr", numR.full(), PB.full(), pa15.full(), ADD)
    sl.free("pa", "pb", "pa15", "pb15")

    rL = sl.alloc_f32("rl", WF, F0)
    nc.vector.reciprocal_approx_fast(out=rL.full(), in_=denL.full())
    rR = sl.alloc_f32("rr", WF, F0)
    nc.vector.reciprocal_approx_fast(out=rR.full(), in_=denR.full())
    sl.free("denl", "denr")
    gL = sl.alloc("gl", WF, F0)   # mL/3 = numL * (1/denL3)
    tt("gl", gL.full(), numL.full(), rL.full(), MUL)
    gR = sl.alloc("gr", WF, F0)   # mR/3
    tt("gr", gR.full(), numR.full(), rR.full(), MUL)
    sl.free("numl", "numr", "rl", "rr")

    # tF = dq[f-1]/6 + dq[f]/3 ; tG = dq[f+1]/6 + dq[f]/3
    tF = sl.alloc("tf", NF, F0)
    tt("tf", tF.full(), dq6.win(F0 - 1, NF), dq3.win(F0, NF), ADD)
    tG = sl.alloc("tg", NF, F0)
    tt("tg", tG.full(), dq6.win(F0 + 1, NF), dq3.win(F0, NF), ADD)
    sl.free("dq6", "dq3")

    # qL = gL + tF + h[f];  qR = h[f+1] - (gR[f+1] + tG)
    aL = sl.alloc("al", NF, F0)
    tt("al", aL.full(), gL.win(F0, NF), tF.full(), ADD)
    qL = sl.alloc("ql", NF, F0)
    tt("ql", qL.full(), aL.full(), H.win(F0, NF), ADD)
    aR = sl.alloc("ar", NF, F0)
    tt("ar", aR.full(), gR.win(F0 + 1, NF), tG.full(), ADD)
    qR = sl.alloc("qr", NF, F0)
    tt("qr", qR.full(), H.win(F0 + 1, NF), aR.full(), SUB)
    sl.free("gl", "gr", "tf", "tg", "al", "ar")

    t1 = sl.alloc("t1", NF, F0)
    tt("t1", t1.full(), UP.win(F0, NF), qL.full(), MUL)
    t2 = sl.alloc("t2", NF, F0)
    tt("t2", t2.full(), UM.win(F0, NF), qR.full(), MUL)
    sl.free("ql", "qr")

    fe = sl.alloc("fe", NF, F0)
    tt("fe", fe.full(), t1.full(), t2.full(), ADD)
    sl.free("t1", "t2")
    return fe


def build_program(rpc=RPC, nx=NX, xc=XC, macb=MACB, reps=1, hw_loop=False,
                  asg=None, work_bufs=2, nslots=18, psum_dma=True):
    """SPMD Bass program computing one core's [rpc, nx] tendency block."""
    assert rpc % 128 == 0 and nx % xc == 0 and nx % (128 * macb) == 0
    asg = dict(DEFAULT_ASG, **(asg or {}))
    yb = rpc // 128
    nchunk = nx // xc
    nmac = nx // (128 * macb)
    mcw = 128 * macb  # macro tile column width
    YW = rpc + 6
    VW = rpc + 1

    nc = bacc.Bacc("TRN2", target_bir_lowering=False, debug=False)
    hs = nc.dram_tensor("hs", [rpc + 6, nx + 6], F32, kind="ExternalInput")
    us = nc.dram_tensor("us", [rpc, nx + 6], F32, kind="ExternalInput")
    vs = nc.dram_tensor("vs", [rpc + 1, nx], F32, kind="ExternalInput")
    out_d = nc.dram_tensor("out", [rpc, nx], F32, kind="ExternalOutput")

    V = nc.vector
    A = nc.scalar

    # const AP for the Act Square(bias=EPSP) beta form
    _epst = nc.alloc_sbuf_tensor("const-epsp", [128, 1], F32)
    nc.gpsimd.memset(_epst.ap(), EPSP)
    nc.const_aps.aps[(F32, EPSP)] = _epst.ap()
    nc.all_engine_barrier()

    with TileContext(nc) as tc:
        with (
            tc.tile_pool(name="const", bufs=1) as cpool,
            tc.tile_pool(name="io", bufs=2) as io,
            tc.tile_pool(name="cast", bufs=2) as cast,
            tc.tile_pool(name="ysrc", bufs=2) as ysrc,
            tc.tile_pool(name="work", bufs=1) as work,
            tc.tile_pool(name="keep", bufs=1) as keep,
            tc.tile_pool(name="outp", bufs=3) as outp,
            tc.tile_pool(name="psum", bufs=2, space="PSUM") as pps,
        ):
            from concourse.masks import make_identity
            ident = cpool.tile([128, 128], F32, name="ident")
            make_identity(nc, ident[:])
            identb = cpool.tile([128, 128], BF16, name="identb")
            A.copy(identb[:], ident[:])

            def body():
                dfx_arrs = {
                    b: _Arr(keep.tile([128, nx], BF16, tag=f"dfx{b}",
                                      name=f"dfx{b}"), 0)
                    for b in range(yb)
                }

                def x_chunk(b, c):
                        dfxb = dfx_arrs[b].tile
                        F0 = c * xc - 1
                        WF = xc + 2
                        bH = c * xc - 3
                        hx = io.tile([128, xc + 6], F32, tag="hx", name="hx")
                        nc.sync.dma_start(
                            out=hx[:],
                            in_=hs[3 + b * 128:3 + b * 128 + 128,
                                   c * xc:c * xc + xc + 6],
                        )
                        ux = io.tile([128, xc + 1], F32, tag="ux", name="ux")
                        nc.sync.dma_start(
                            out=ux[:],
                            in_=us[b * 128:b * 128 + 128,
                                   c * xc + 2:c * xc + xc + 3],
                        )
                        HB = _Arr(cast.tile([128, xc + 6], BF16, tag="hbf",
                                            name="hbf"), bH)
                        A.copy(HB.full(), hx[:])
                        ub = cast.tile([128, xc + 1], BF16, tag="ubf",
                                       name="ubf")
                        A.copy(ub[:], ux[:])
                        UP = _Arr(cast.tile([128, xc + 1], BF16, tag="upx",
                                            name="upx"), F0)
                        V.tensor_scalar(UP.full(), ub[:], 0.0, -1.0 / DX,
                                        MAX, MUL)
                        UM = _Arr(cast.tile([128, xc + 1], BF16, tag="umx",
                                            name="umx"), F0)
                        V.tensor_scalar(UM.full(), ub[:], 0.0, -1.0 / DX,
                                        MIN, MUL)
                        sl = _Slots(work, [128, 0], nslots=nslots,
                                    bufs=work_bufs)
                        fe = _flux_v2(nc, sl, HB, UP, UM, F0, WF, xc + 1, asg)
                        # dfx = fe'[f] - fe'[f-1]  (already -flux/DX scaled)
                        V.tensor_tensor(
                            dfxb[:, c * xc:(c + 1) * xc],
                            fe.win(c * xc, xc), fe.win(c * xc - 1, xc), SUB)
                        sl.free("fe")

                def y_mac(mac):
                    HT = cast.tile([128, macb, YW], BF16, tag="ht", name="ht")
                    VTb = cast.tile([128, macb, VW], BF16, tag="vt",
                                    name="vt")
                    hsrc = {}
                    vsrc = {}
                    for k, roff in enumerate(range(0, YW, 128)):
                        rlen = min(128, YW - roff)
                        t = ysrc.tile([128, mcw], F32, tag=f"hsrc{k}",
                                      name=f"hsrc{k}")
                        nc.sync.dma_start(
                            out=t[:rlen, :],
                            in_=hs[roff:roff + rlen,
                                   3 + mac * mcw:3 + (mac + 1) * mcw],
                        )
                        hsrc[roff] = (t, rlen)
                    for k, roff in enumerate(range(0, VW, 128)):
                        rlen = min(128, VW - roff)
                        t = ysrc.tile([128, mcw], F32, tag=f"vsrc{k}",
                                      name=f"vsrc{k}")
                        nc.sync.dma_start(
                            out=t[:rlen, :],
                            in_=vs[roff:roff + rlen,
                                   mac * mcw:(mac + 1) * mcw],
                        )
                        vsrc[roff] = (t, rlen)
                    for bb in range(macb):
                        hp = pps.tile([128, YW], F32, tag="hps", name="hps")
                        for roff, (t, rlen) in hsrc.items():
                            nc.tensor.transpose(
                                hp[:, roff:roff + rlen],
                                t[:rlen, bb * 128:(bb + 1) * 128],
                                ident[:rlen, :rlen],
                            )
                        A.copy(HT[:, bb, :], hp[:])
                        vp = pps.tile([128, VW], F32, tag="vps", name="vps")
                        for roff, (t, rlen) in vsrc.items():
                            nc.tensor.transpose(
                                vp[:, roff:roff + rlen],
                                t[:rlen, bb * 128:(bb + 1) * 128],
                                ident[:rlen, :rlen],
                            )
                        A.copy(VTb[:, bb, :], vp[:])

                    UPy = _Arr(cast.tile([128, macb, VW], BF16, tag="upy",
                                         name="upy"), 0)
                    V.tensor_scalar(UPy.full(), VTb[:], 0.0, -1.0 / DY,
                                    MAX, MUL)
                    UMy = _Arr(cast.tile([128, macb, VW], BF16, tag="umy",
                                         name="umy"), 0)
                    V.tensor_scalar(UMy.full(), VTb[:], 0.0, -1.0 / DY,
                                    MIN, MUL)

                    Hy = _Arr(HT, -2)
                    sl = _Slots(work, [128, macb, 0], nslots=nslots,
                                bufs=work_bufs)
                    fn = _flux_v2(nc, sl, Hy, UPy, UMy, 0, rpc + 2, rpc + 1,
                                  asg)
                    dfy = work.tile([128, macb, rpc], BF16, tag="dfy",
                                    name="dfy", bufs=work_bufs)
                    V.tensor_tensor(dfy[:], fn.win(1, rpc),
                                    fn.win(0, rpc), SUB)
                    sl.free("fe")

                    for b in range(yb):
                        zyp = pps.tile([128, mcw], BF16, tag="zyp",
                                       name="zyp")
                        for bb in range(macb):
                            nc.tensor.transpose(
                                zyp[:, bb * 128:(bb + 1) * 128],
                                dfy[:, bb, b * 128:b * 128 + 128],
                                identb[:],
                            )
                        ot = outp.tile([128, mcw], F32, tag="outsb",
                                       name="outsb")
                        V.tensor_tensor(
                            ot[:],
                            dfx_arrs[b].tile[:, mac * mcw:(mac + 1) * mcw],
                            zyp[:], ADD)
                        nc.sync.dma_start(
                            out=out_d[b * 128:b * 128 + 128,
                                      mac * mcw:(mac + 1) * mcw],
                            in_=ot[:],
                        )

                for c in range(nchunk):
                    for b in range(yb):
                        x_chunk(b, c)
                    for mac in range(c * xc // mcw, (c + 1) * xc // mcw):
                        y_mac(mac)

            if hw_loop:
                with tc.For_i(0, reps):
                    body()
            else:
                for _ in range(reps):
                    body()
    nc.compile()
    return nc


# --------------------------------------------------------------------------
# Host side
# --------------------------------------------------------------------------
def make_shards(h, u, v, n_cores=N_CORES):
    rpc = h.shape[0] // n_cores
    hp = np.pad(h, ((3, 3), (3, 3)), mode="edge")
    up = np.pad(u, ((0, 0), (3, 3)), mode="edge")
    vp = np.pad(v, ((3, 3), (0, 0)), mode="edge")
    maps = []
    for i in range(n_cores):
        r0 = i * rpc
        maps.append({
            "hs": np.ascontiguousarray(hp[r0:r0 + rpc + 6, :]),
            "us": np.ascontiguousarray(up[r0:r0 + rpc, :]),
            "vs": np.ascontiguousarray(vp[r0 + 2:r0 + 2 + rpc + 1, :]),
        })
    return maps


_NC_CACHE = {}


def kernel(h, u, v):
    h = np.asarray(h, dtype=np.float32)
    u = np.asarray(u, dtype=np.float32)
    v = np.asarray(v, dtype=np.float32)
    assert h.shape == (NY, NX), h.shape

    from concourse.bass_utils import run_bass_kernel_spmd

    if "main" not in _NC_CACHE:
        _NC_CACHE["main"] = build_program()
    nc = _NC_CACHE["main"]

    in_maps = make_shards(h, u, v)
    res = run_bass_kernel_spmd(nc, in_maps, list(range(N_CORES)))
    out = np.concatenate([res.results[i]["out"] for i in range(N_CORES)],
                         axis=0)
    out[:2, :] = 0.0
    out[-2:, :] = 0.0
    out[:, :2] = 0.0
    out[:, -2:] = 0.0
    return out
